# revision 24
# baseline (speedup 1.0000x reference)
"""Trainium2 kernel for nn_BlockSparseMatrix: block-sparse -> dense reconstruction
plus CSR/CSC index building.

Strategy (8 NeuronCores, SPMD):
  - Shard the 256x256 block grid by block-row: core m owns block-rows
    [32m, 32m+32) = dense rows [1024m, 1024m+1024), a 32 MiB output stripe.
    Since nnz positions are sorted row-major, each core's blocks are a
    contiguous slice of `data`.
  - The stripe is built in 8 tiles of (128 partitions x 8192 f32) = 4 block
    rows each; each tile splits into 4 "units" of 2 column-windows; each
    window is 128 grid slots (4 block-rows g x 32 block-cols c').  Per unit:
      1. zero-fill a (128, 2048) staging tile (ACT),
      2. two indirect-DMA gathers (one per window): partition 32g+c'
         receives the whole 4 KiB block for slot (g, c') from DRAM, one
         full-rate descriptor per occupied slot; empty slots carry an OOB
         index and are skipped, leaving zeros.  The per-partition dynamic
         source address is what makes placement data-dependent, so the
         instruction stream is mask-independent,
      3. per window, one DVE stream-transpose (32x32 blocks) whose permuted
         output AP swaps the within-32 partition coordinate c' with the
         within-32 free coordinate q AND swaps the free halves (b, c') ->
         (c', b): the transposed block lands directly at its column slot,
         with partition = dense row.  (A quarter of the units instead do a
         contiguous transpose + strided ACT copy to balance DVE/ACT load.)
      4. one (128, 2048) HWDGE DMA writes the unit to the dense stripe
         (per-row 8 KiB contiguous descriptors).
  - The tiny index outputs (CSR/CSC pointers, <1 MB total) are computed on
    host; the 256 MiB dense tensor dominates all memory traffic and is
    produced on device.

HBM traffic per core: ~8.5 MiB block reads + 32 MiB dense writes; measured
~145 us against a ~113 us HBM roofline (the two cores of an HBM stack pair
saturate their 716 GB/s stack).
"""

import sys
from contextlib import ExitStack

import numpy as np

for _p in (
    "/root/.axon_site",
    "/root/.axon_site/_ro/trn_rl_repo",
    "/root/.axon_site/_ro/pypackages",
):
    if _p not in sys.path:
        sys.path.append(_p)

BH = BW = 32
X = Y = 256
M = 8                           # cores
ROWS_PER_CORE = X // M          # 32 block-rows
TILES = ROWS_PER_CORE // 4      # 8 tiles of 4 block-rows
WINDOWS = Y // 32               # 8 col-windows of 32 block-cols per tile
GROUPS = TILES * WINDOWS        # 64 gather windows per core
F = Y * BW                      # 8192 f32 free dim per output tile
K_MAX_DEFAULT = 2560            # padded blocks per core stripe (marker = K_MAX)

_programs: dict = {}

# test harness hooks: extra kwargs for run_bass_kernel_spmd and the last results
run_kwargs: dict = {}
last_results = None


def _build_body(tc, dense_ap, blk_ap, idx_ap, ctx: ExitStack, k_max: int):
    import concourse.bass as bass
    import concourse.mybir as mybir

    nc = tc.nc
    f32 = mybir.dt.float32
    idxp = ctx.enter_context(tc.tile_pool(name="idxp", bufs=1))
    zp = ctx.enter_context(tc.tile_pool(name="zp", bufs=1))
    sp = ctx.enter_context(tc.tile_pool(name="sp", bufs=8))
    rp = ctx.enter_context(tc.tile_pool(name="rp", bufs=4))
    tp = ctx.enter_context(tc.tile_pool(name="tp", bufs=8))

    idx_sb = idxp.tile([128, GROUPS], mybir.dt.int32)
    nc.sync.dma_start(idx_sb[:], idx_ap[:])

    zero = zp.tile([128, 2048], f32)
    nc.vector.memset(zero[:], 0.0)

    for t in range(TILES):
        for w2 in range(WINDOWS // 2):
            gi = t * WINDOWS + 2 * w2
            stg = sp.tile([128, 2048], f32, tag="stg")
            # zero-fill on ACT (otherwise idle); Pool must stay gather-only.
            # DVE covers the first units so the pipeline fills before ACT's
            # table load completes.
            if t == 0 and w2 < 2:
                nc.vector.memset(stg[:], 0.0)
            else:
                nc.scalar.copy(stg[:], zero[:])
            # dynamic block placement: partition 32g+c' <- block at
            # (block-row 4t+g, col 32w+c'), one 4 KiB descriptor per slot
            for h in range(2):
                nc.gpsimd.indirect_dma_start(
                    out=stg[:, 1024 * h:1024 * (h + 1)],
                    out_offset=None,
                    in_=blk_ap[:],
                    in_offset=bass.IndirectOffsetOnAxis(
                        ap=idx_sb[:, gi + h:gi + h + 1], axis=0
                    ),
                    bounds_check=k_max - 1,
                    oob_is_err=False,
                )
            # stream-transpose with permuted out AP: writes the transposed
            # block of slot (g, c') directly at its column slot in W.  The
            # strided write costs ~1.6x on DVE, so a quarter of the units do
            # a contiguous transpose + strided ACT copy instead.
            W = tp.tile([128, 2048], f32, tag="W")
            if w2 % 4 == 1 and t < 7:
                R = rp.tile([128, 2048], f32, tag="R")
                for h in range(2):
                    sl = slice(1024 * h, 1024 * (h + 1))
                    nc.vector.transpose(R[:, sl], stg[:, sl])
                    nc.scalar.copy(
                        W[:, sl].rearrange("p (c b) -> p b c", b=32),
                        R[:, sl].rearrange("p (b c) -> p b c", c=32),
                    )
            else:
                nc.vector.transpose(
                    W[:].rearrange("p (h c b) -> p h b c", h=2, b=32),
                    stg[:],
                )
            nc.sync.dma_start(
                dense_ap[128 * t:128 * (t + 1), 2048 * w2:2048 * (w2 + 1)],
                W[:],
            )


def _get_program(k_max: int):
    if k_max in _programs:
        return _programs[k_max]

    import concourse.bacc as bacc
    import concourse.mybir as mybir
    import concourse.tile as tile

    nc = bacc.Bacc("TRN2", target_bir_lowering=False, debug=False, num_devices=M)
    blk = nc.dram_tensor(
        "blk", [k_max, BH * BW], mybir.dt.float32, kind="ExternalInput"
    ).ap()
    idx = nc.dram_tensor(
        "idx", [128, GROUPS], mybir.dt.int32, kind="ExternalInput"
    ).ap()
    dense = nc.dram_tensor(
        "dense", [ROWS_PER_CORE * BH, F], mybir.dt.float32, kind="ExternalOutput"
    ).ap()

    with tile.TileContext(nc) as tc, ExitStack() as ctx:
        _build_body(tc, dense, blk, idx, ctx, k_max)
    nc.compile()
    _programs[k_max] = nc
    return nc


def _size_n(arrs, n):
    """Mirror jnp.nonzero(..., size=n): truncate or zero-pad each array."""
    out = []
    for a in arrs:
        if len(a) >= n:
            out.append(a[:n])
        else:
            out.append(np.concatenate([a, np.zeros(n - len(a), a.dtype)]))
    return out


def _host_indices(mask: np.ndarray, n: int):
    """Everything except the dense tensor, mirroring reference() on host."""
    i32 = np.int32
    nz_exact = np.nonzero(mask)  # row-major order
    rows, cols = _size_n(nz_exact, n)
    block_ptr = np.arange(n)

    blocks = np.stack([cols, rows], axis=1).reshape(-1).astype(i32)

    row_counts = np.zeros(X + 1, np.int64)
    np.add.at(row_counts, rows + 1, 1)
    row_start_ends_a = np.cumsum(row_counts).astype(i32)
    cols_a = np.stack([cols, block_ptr], axis=1).astype(i32)

    bi = np.zeros(X * Y, np.int64)
    bi[rows * Y + cols] = block_ptr + 1
    bit = bi.reshape(X, Y).T.reshape(-1)
    (tpos,) = _size_n(np.nonzero(bit), n)
    block_ptr_t = (bit[tpos] - 1).astype(i32)

    rows_t, cols_t = _size_n(np.nonzero(mask.T), n)
    col_counts = np.zeros(Y + 1, np.int64)
    np.add.at(col_counts, rows_t + 1, 1)
    col_start_ends_b = np.cumsum(col_counts).astype(i32)
    rows_b = np.stack([cols_t, block_ptr_t], axis=1).astype(i32)

    exact = len(nz_exact[0]) == n
    return (
        rows, cols, blocks, cols_a, row_start_ends_a, rows_b,
        col_start_ends_b, exact,
    )


def _shard_inputs(rows, cols, data, k_max):
    """Per-core (blk, idx) arrays."""
    in_maps = []
    stripe_bounds = np.searchsorted(rows, np.arange(M + 1) * ROWS_PER_CORE)
    for m in range(M):
        s, e = int(stripe_bounds[m]), int(stripe_bounds[m + 1])
        k = e - s
        blk = np.zeros((k_max, BH * BW), np.float32)
        blk[:k] = data[s * BH:e * BH].reshape(k, BH * BW)

        grid = np.full((ROWS_PER_CORE, Y), k_max, np.int64)
        grid[rows[s:e] - m * ROWS_PER_CORE, cols[s:e]] = np.arange(k)
        # idx[32g + c', 8t + w] = grid[4t + g, 32w + c'] (or k_max marker)
        g4 = grid.reshape(TILES, 4, WINDOWS, 32)  # [t, g, w, c']
        idx = g4.transpose(1, 3, 0, 2).reshape(128, GROUPS).astype(np.int32)
        in_maps.append({"blk": blk, "idx": idx})
    return in_maps


def _dense_numpy(mask, data, n):
    """Host fallback for out-of-contract inputs (nnz != n)."""
    import jax
    import jax.numpy as jnp

    with jax.default_device(jax.devices("cpu")[0]):
        rows, cols = jnp.nonzero(jnp.asarray(mask), size=n)
        d = jnp.asarray(data).reshape(n, BH, BW)
        dense = jnp.zeros((X, BH, Y, BW), d.dtype)
        dense = dense.at[rows, :, cols, :].set(jnp.swapaxes(d, 1, 2))
        return np.asarray(dense.reshape(X * BH, Y * BW))


def kernel(block_mask, data):
    global last_results
    mask = np.asarray(block_mask, dtype=bool)
    data = np.asarray(data, dtype=np.float32)
    n = data.shape[0] // BH

    (
        rows,
        cols,
        blocks,
        cols_a,
        row_start_ends_a,
        rows_b,
        col_start_ends_b,
        exact,
    ) = _host_indices(mask, n)

    if not exact:
        # out-of-contract input (nnz != n): host fallback for the dense part
        dense = _dense_numpy(mask, data, n)
        return dense, blocks, cols_a, row_start_ends_a, rows_b, col_start_ends_b

    k_need = int(
        np.max(np.diff(np.searchsorted(rows, np.arange(M + 1) * ROWS_PER_CORE)))
    )
    k_max = K_MAX_DEFAULT if k_need < K_MAX_DEFAULT else -(-(k_need + 1) // 512) * 512

    in_maps = _shard_inputs(rows, cols, data, k_max)
    nc = _get_program(k_max)

    from concourse.bass_utils import run_bass_kernel_spmd

    res = None
    for attempt in range(3):
        try:
            res = run_bass_kernel_spmd(
                nc, in_maps, core_ids=list(range(M)), **run_kwargs
            )
            break
        except Exception:
            # rare transient NRT_EXEC_UNIT_UNRECOVERABLE; retry
            if attempt == 2:
                raise
    last_results = res
    dense = np.concatenate([r["dense"] for r in res.results], axis=0)

    return dense, blocks, cols_a, row_start_ends_a, rows_b, col_start_ends_b


# revision 25
# speedup vs baseline: 1.0690x; 1.0690x over previous
"""Trainium2 kernel for nn_BlockSparseMatrix: block-sparse -> dense reconstruction
plus CSR/CSC index building.

Strategy (8 NeuronCores, SPMD):
  - Shard the 256x256 block grid by block-row: core m owns block-rows
    [32m, 32m+32) = dense rows [1024m, 1024m+1024), a 32 MiB output stripe.
    Since nnz positions are sorted row-major, each core's blocks are a
    contiguous slice of `data`.
  - The stripe is built in 8 tiles of (128 partitions x 8192 f32) = 4 block
    rows each; each tile splits into 4 "units" of 2 column-windows; each
    window is 128 grid slots (4 block-rows g x 32 block-cols c').  Per unit:
      1. zero-fill a (128, 2048) staging tile (ACT),
      2. two indirect-DMA gathers (one per window): partition 32g+c'
         receives the whole 4 KiB block for slot (g, c') from DRAM, one
         full-rate descriptor per occupied slot; empty slots carry an OOB
         index and are skipped, leaving zeros.  The per-partition dynamic
         source address is what makes placement data-dependent, so the
         instruction stream is mask-independent,
      3. per window, one DVE stream-transpose (32x32 blocks) whose permuted
         output AP swaps the within-32 partition coordinate c' with the
         within-32 free coordinate q AND swaps the free halves (b, c') ->
         (c', b): the transposed block lands directly at its column slot,
         with partition = dense row.  (A quarter of the units instead do a
         contiguous transpose + strided ACT copy to balance DVE/ACT load.)
      4. one (128, 2048) HWDGE DMA writes the unit to the dense stripe
         (per-row 8 KiB contiguous descriptors).
  - The tiny index outputs (CSR/CSC pointers, <1 MB total) are computed on
    host; the 256 MiB dense tensor dominates all memory traffic and is
    produced on device.

HBM traffic per core: ~8.5 MiB block reads + 32 MiB dense writes; measured
~145 us against a ~113 us HBM roofline (the two cores of an HBM stack pair
saturate their 716 GB/s stack).
"""

import sys
from contextlib import ExitStack

import numpy as np

for _p in (
    "/root/.axon_site",
    "/root/.axon_site/_ro/trn_rl_repo",
    "/root/.axon_site/_ro/pypackages",
):
    if _p not in sys.path:
        sys.path.append(_p)

BH = BW = 32
X = Y = 256
M = 8                           # cores
ROWS_PER_CORE = X // M          # 32 block-rows
TILES = ROWS_PER_CORE // 4      # 8 tiles of 4 block-rows
WINDOWS = Y // 32               # 8 col-windows of 32 block-cols per tile
GROUPS = TILES * WINDOWS        # 64 gather windows per core
F = Y * BW                      # 8192 f32 free dim per output tile
K_MAX_DEFAULT = 2560            # padded blocks per core stripe (marker = K_MAX)

_programs: dict = {}

# test harness hooks: extra kwargs for run_bass_kernel_spmd and the last results
run_kwargs: dict = {}
last_results = None


def _build_body(tc, dense_ap, blk_ap, idx_ap, ctx: ExitStack, k_max: int):
    import concourse.bass as bass
    import concourse.mybir as mybir

    nc = tc.nc
    f32 = mybir.dt.float32
    idxp = ctx.enter_context(tc.tile_pool(name="idxp", bufs=1))
    zp = ctx.enter_context(tc.tile_pool(name="zp", bufs=1))
    sp = ctx.enter_context(tc.tile_pool(name="sp", bufs=9))
    rp = ctx.enter_context(tc.tile_pool(name="rp", bufs=4))
    tp = ctx.enter_context(tc.tile_pool(name="tp", bufs=9))

    idx_sb = idxp.tile([128, GROUPS], mybir.dt.int32)
    nc.sync.dma_start(idx_sb[:], idx_ap[:])

    zero = zp.tile([128, 2048], f32)
    nc.vector.memset(zero[:], 0.0)

    for t in range(TILES):
        for w2 in range(WINDOWS // 2):
            gi = t * WINDOWS + 2 * w2
            stg = sp.tile([128, 2048], f32, tag="stg")
            # zero-fill on ACT (otherwise idle); Pool must stay gather-only.
            # DVE covers the first units so the pipeline fills before ACT's
            # table load completes.
            if t == 0 and w2 < 2:
                nc.vector.memset(stg[:], 0.0)
            else:
                nc.scalar.copy(stg[:], zero[:])
            # dynamic block placement: partition 32g+c' <- block at
            # (block-row 4t+g, col 32w+c'), one 4 KiB descriptor per slot
            for h in range(2):
                nc.gpsimd.indirect_dma_start(
                    out=stg[:, 1024 * h:1024 * (h + 1)],
                    out_offset=None,
                    in_=blk_ap[:],
                    in_offset=bass.IndirectOffsetOnAxis(
                        ap=idx_sb[:, gi + h:gi + h + 1], axis=0
                    ),
                    bounds_check=k_max - 1,
                    oob_is_err=False,
                )
            # stream-transpose with permuted out AP: writes the transposed
            # block of slot (g, c') directly at its column slot in W.  The
            # strided write costs ~1.6x on DVE, so a quarter of the units do
            # a contiguous transpose + strided ACT copy instead.
            W = tp.tile([128, 2048], f32, tag="W")
            if w2 % 4 == 1 and t < 7:
                R = rp.tile([128, 2048], f32, tag="R")
                for h in range(2):
                    sl = slice(1024 * h, 1024 * (h + 1))
                    nc.vector.transpose(R[:, sl], stg[:, sl])
                    nc.scalar.copy(
                        W[:, sl].rearrange("p (c b) -> p b c", b=32),
                        R[:, sl].rearrange("p (b c) -> p b c", c=32),
                    )
            else:
                for h in range(2):
                    sl = slice(1024 * h, 1024 * (h + 1))
                    nc.vector.transpose(
                        W[:, sl].rearrange("p (c b) -> p b c", b=32),
                        stg[:, sl],
                    )
            nc.sync.dma_start(
                dense_ap[128 * t:128 * (t + 1), 2048 * w2:2048 * (w2 + 1)],
                W[:],
            )


def _get_program(k_max: int):
    if k_max in _programs:
        return _programs[k_max]

    import concourse.bacc as bacc
    import concourse.mybir as mybir
    import concourse.tile as tile

    nc = bacc.Bacc("TRN2", target_bir_lowering=False, debug=False, num_devices=M)
    blk = nc.dram_tensor(
        "blk", [k_max, BH * BW], mybir.dt.float32, kind="ExternalInput"
    ).ap()
    idx = nc.dram_tensor(
        "idx", [128, GROUPS], mybir.dt.int32, kind="ExternalInput"
    ).ap()
    dense = nc.dram_tensor(
        "dense", [ROWS_PER_CORE * BH, F], mybir.dt.float32, kind="ExternalOutput"
    ).ap()

    with tile.TileContext(nc) as tc, ExitStack() as ctx:
        _build_body(tc, dense, blk, idx, ctx, k_max)
    nc.compile()
    _programs[k_max] = nc
    return nc


def _size_n(arrs, n):
    """Mirror jnp.nonzero(..., size=n): truncate or zero-pad each array."""
    out = []
    for a in arrs:
        if len(a) >= n:
            out.append(a[:n])
        else:
            out.append(np.concatenate([a, np.zeros(n - len(a), a.dtype)]))
    return out


def _host_indices(mask: np.ndarray, n: int):
    """Everything except the dense tensor, mirroring reference() on host."""
    i32 = np.int32
    nz_exact = np.nonzero(mask)  # row-major order
    rows, cols = _size_n(nz_exact, n)
    block_ptr = np.arange(n)

    blocks = np.stack([cols, rows], axis=1).reshape(-1).astype(i32)

    row_counts = np.zeros(X + 1, np.int64)
    np.add.at(row_counts, rows + 1, 1)
    row_start_ends_a = np.cumsum(row_counts).astype(i32)
    cols_a = np.stack([cols, block_ptr], axis=1).astype(i32)

    bi = np.zeros(X * Y, np.int64)
    bi[rows * Y + cols] = block_ptr + 1
    bit = bi.reshape(X, Y).T.reshape(-1)
    (tpos,) = _size_n(np.nonzero(bit), n)
    block_ptr_t = (bit[tpos] - 1).astype(i32)

    rows_t, cols_t = _size_n(np.nonzero(mask.T), n)
    col_counts = np.zeros(Y + 1, np.int64)
    np.add.at(col_counts, rows_t + 1, 1)
    col_start_ends_b = np.cumsum(col_counts).astype(i32)
    rows_b = np.stack([cols_t, block_ptr_t], axis=1).astype(i32)

    exact = len(nz_exact[0]) == n
    return (
        rows, cols, blocks, cols_a, row_start_ends_a, rows_b,
        col_start_ends_b, exact,
    )


def _shard_inputs(rows, cols, data, k_max):
    """Per-core (blk, idx) arrays."""
    in_maps = []
    stripe_bounds = np.searchsorted(rows, np.arange(M + 1) * ROWS_PER_CORE)
    for m in range(M):
        s, e = int(stripe_bounds[m]), int(stripe_bounds[m + 1])
        k = e - s
        blk = np.zeros((k_max, BH * BW), np.float32)
        blk[:k] = data[s * BH:e * BH].reshape(k, BH * BW)

        grid = np.full((ROWS_PER_CORE, Y), k_max, np.int64)
        grid[rows[s:e] - m * ROWS_PER_CORE, cols[s:e]] = np.arange(k)
        # idx[32g + c', 8t + w] = grid[4t + g, 32w + c'] (or k_max marker)
        g4 = grid.reshape(TILES, 4, WINDOWS, 32)  # [t, g, w, c']
        idx = g4.transpose(1, 3, 0, 2).reshape(128, GROUPS).astype(np.int32)
        in_maps.append({"blk": blk, "idx": idx})
    return in_maps


def _dense_numpy(mask, data, n):
    """Host fallback for out-of-contract inputs (nnz != n)."""
    import jax
    import jax.numpy as jnp

    with jax.default_device(jax.devices("cpu")[0]):
        rows, cols = jnp.nonzero(jnp.asarray(mask), size=n)
        d = jnp.asarray(data).reshape(n, BH, BW)
        dense = jnp.zeros((X, BH, Y, BW), d.dtype)
        dense = dense.at[rows, :, cols, :].set(jnp.swapaxes(d, 1, 2))
        return np.asarray(dense.reshape(X * BH, Y * BW))


def kernel(block_mask, data):
    global last_results
    mask = np.asarray(block_mask, dtype=bool)
    data = np.asarray(data, dtype=np.float32)
    n = data.shape[0] // BH

    (
        rows,
        cols,
        blocks,
        cols_a,
        row_start_ends_a,
        rows_b,
        col_start_ends_b,
        exact,
    ) = _host_indices(mask, n)

    if not exact:
        # out-of-contract input (nnz != n): host fallback for the dense part
        dense = _dense_numpy(mask, data, n)
        return dense, blocks, cols_a, row_start_ends_a, rows_b, col_start_ends_b

    k_need = int(
        np.max(np.diff(np.searchsorted(rows, np.arange(M + 1) * ROWS_PER_CORE)))
    )
    k_max = K_MAX_DEFAULT if k_need < K_MAX_DEFAULT else -(-(k_need + 1) // 512) * 512

    in_maps = _shard_inputs(rows, cols, data, k_max)
    nc = _get_program(k_max)

    from concourse.bass_utils import run_bass_kernel_spmd

    res = None
    for attempt in range(3):
        try:
            res = run_bass_kernel_spmd(
                nc, in_maps, core_ids=list(range(M)), **run_kwargs
            )
            break
        except Exception:
            # rare transient NRT_EXEC_UNIT_UNRECOVERABLE; retry
            if attempt == 2:
                raise
    last_results = res
    dense = np.concatenate([r["dense"] for r in res.results], axis=0)

    return dense, blocks, cols_a, row_start_ends_a, rows_b, col_start_ends_b
